# revision 8
# baseline (speedup 1.0000x reference)
"""MoE layer (top-2 of 24 experts, d_model=1024, d_ff=4096, T=4096 tokens)
on 8 Trainium2 NeuronCores.

Strategy (expert-parallel, host-routed):
  - Host computes the gate, top-2 ids and softmax probs, gathers each
    expert's tokens into a transposed buffer xT[e] = [D, C_j].
  - Experts sharded 3 per core, balanced by token count (sorted-deal into
    3 slots); slot capacity = max count in that octile, 16-aligned exact.
  - Per expert on device (all matmul free dims = exact token count C):
      phase A: ht[m] = gelu(w1_km.T @ xT + b1)    32 m-groups, N=C
      phase B (transposed): 8 PSUM-resident banks pb[md] = yT d-chunks,
               k-outer sweep so w2 streams HBM exactly once:
               pb[md] += w2_k[:, md].T @ ht[k]    N=C, no 128-padding
      drain:   yT[md] = pb[md] * prob_broadcast   (DVE), DMA out
  - All matmul operand tiles are narrow ([P, <=1024], 2KB rows) -- wide
    packed tiles measurably slow the PE weight path (+30ns/matmul).
  - Weight DMAs are deadline-sorted on two HWDGE rings (sync: w1,
    scalar: x/w2/y); pool buffer recycling provides runtime pacing.
  - ~28 warmup matmuls on a zeroed scratch tile keep the PE HAM clock
    un-throttled (K=8/8) through the startup DMA window.
  - Host scatters the two per-token expert outputs back together.

Matmuls in bf16 with fp32 PSUM accumulation (rel err ~3e-3); b1 applied
exactly as the ACT per-partition bias.
"""

import numpy as np
import ml_dtypes

P = 128
D_MODEL = 1024
D_FF = 4096
NUM_EXPERTS = 24
TOP_K = 2
N_CORES = 8
E_LOC = NUM_EXPERTS // N_CORES   # 3 experts per core
KD = D_MODEL // P                # 8  k-chunks over d_model
KF = D_FF // P                   # 32 k-chunks over d_ff
MD = D_MODEL // P                # 8  output d-chunks (phase B)
W1C = 4                          # w1 column chunks per k-tile
W1CW = D_FF // W1C               # 1024 columns per chunk
MPC = W1CW // P                  # 8 m-tiles per chunk
BF16 = ml_dtypes.bfloat16
NWARM = 28                       # PE warmup matmuls


def _build(Cs, repeat=1):
    """Per-core Bass program (SPMD: same program, per-core data).

    Cs: per-slot token capacities (16-aligned, each <= 512).
    """
    import concourse.bacc as bacc
    import concourse.mybir as mybir
    from concourse.tile import TileContext

    dt = mybir.dt.bfloat16
    f32 = mybir.dt.float32
    mult = mybir.AluOpType.mult
    CT = sum(Cs)
    offs = [sum(Cs[:j]) for j in range(E_LOC)]

    # model timeline (us) for DMA deadline sorting
    tA = [256.0 * C / 2400.0 for C in Cs]
    t0 = []
    t = 0.0
    for j in range(E_LOC):
        t0.append(t)
        t += 2.0 * tA[j]

    # (deadline, ring, kind, e, k, q)  ring 0=sync(w1) 1=scalar(w2/xt/prb)
    events = []
    for e in range(E_LOC):
        for q in range(W1C):
            for k in range(KD):
                events.append((t0[e] + tA[e] * q / W1C - 12.0, 0,
                               'w1', e, k, q))
        for k in range(KF):
            events.append((t0[e] + tA[e] * (1.0 + k / KF) - 8.0, 1,
                           'w2', e, k, 0))
        if e > 0:
            for k in range(KD):
                events.append((t0[e] - 20.0, 1, 'xt', e, k, 0))
    events.append((t0[0] + tA[0] + 5.0, 1, 'prb', 0, 0, 0))
    events.sort(key=lambda ev: (ev[0], ev[1]))

    nc = bacc.Bacc(None, target_bir_lowering=False)
    xt_d = [nc.dram_tensor(f"xt{j}", [KD, P, Cs[j]], dt, kind="ExternalInput")
            for j in range(E_LOC)]
    w1 = nc.dram_tensor("w1", [E_LOC, KD, W1C, P, W1CW], dt,
                        kind="ExternalInput")
    w2 = nc.dram_tensor("w2", [E_LOC, KF, P, D_MODEL], dt,
                        kind="ExternalInput")
    prb = nc.dram_tensor("prb", [P, CT], f32, kind="ExternalInput")
    b1 = nc.dram_tensor("b1", [P, E_LOC * KF], f32, kind="ExternalInput")
    y_d = [nc.dram_tensor(f"y{j}", [D_MODEL, Cs[j]], f32,
                          kind="ExternalOutput") for j in range(E_LOC)]

    with TileContext(nc) as tc:
        with tc.tile_pool(name="consts", bufs=1) as consts, \
             tc.tile_pool(name="xtp", bufs=E_LOC * KD) as xtp, \
             tc.tile_pool(name="w1p", bufs=40) as w1p, \
             tc.tile_pool(name="w2p", bufs=16) as w2p, \
             tc.tile_pool(name="htp", bufs=KF) as htp, \
             tc.tile_pool(name="outp", bufs=3) as outp, \
             tc.tile_pool(name="psp", bufs=8, space="PSUM") as psp:

            # startup loads + PE warmup (keeps HAM at K=8/8 through the
            # initial DMA window; zeroed scratch, results never read)
            xts = {}
            for k in range(KD):
                t_ = xtp.tile([P, Cs[0]], dt, tag="xt0", name="xt0")
                nc.scalar.dma_start(t_[:], xt_d[0][k, :, :])
                xts[(0, k)] = t_
            b1_t = consts.tile([P, E_LOC * KF], f32, tag="b1")
            nc.scalar.dma_start(b1_t[:], b1[:, :])
            wsc = consts.tile([P, 512], dt, tag="wsc")
            nc.vector.memset(wsc[:], 0.0)
            wps = psp.tile([P, 512], f32, tag="ps")
            for _ in range(NWARM):
                nc.tensor.matmul(wps[:], wsc[:, :P], wsc[:],
                                 start=True, stop=True)

            w1ts = {}
            w2ts = {}
            prb_t = [None]
            cur = [0]

            def emit_until(tnow):
                while cur[0] < len(events) and events[cur[0]][0] <= tnow:
                    _, _, kind, e, k, q = events[cur[0]]
                    cur[0] += 1
                    if kind == 'w1':
                        t_ = w1p.tile([P, W1CW], dt, tag="w1", name="w1t")
                        nc.sync.dma_start(t_[:], w1[e, k, q, :, :])
                        w1ts[(e, k, q)] = t_
                    elif kind == 'w2':
                        t_ = w2p.tile([P, D_MODEL], dt, tag="w2", name="w2t")
                        nc.scalar.dma_start(t_[:], w2[e, k, :, :])
                        w2ts[(e, k)] = t_
                    elif kind == 'xt':
                        t_ = xtp.tile([P, Cs[e]], dt, tag=f"xt{e}",
                                      name=f"xt{e}")
                        nc.scalar.dma_start(t_[:], xt_d[e][k, :, :])
                        xts[(e, k)] = t_
                    else:
                        t_ = consts.tile([P, CT], f32, tag="prb", name="prb")
                        nc.scalar.dma_start(t_[:], prb[:, :])
                        prb_t[0] = t_

            mt = 0.0
            for _ in range(repeat):
                for e in range(E_LOC):
                    C = Cs[e]
                    # phase A: ht[m] = gelu(w1.T @ x + b1)  [P dff x C tok]
                    hts = []
                    for m in range(KF):
                        emit_until(mt)
                        pa = psp.tile([P, 512], f32, tag="ps", name="pa")
                        for k in range(KD):
                            nc.tensor.matmul(
                                pa[:, :C],
                                w1ts[(e, k, m // MPC)][:, (m % MPC) * P:
                                                       (m % MPC + 1) * P],
                                xts[(e, k)][:, :],
                                start=(k == 0), stop=(k == KD - 1))
                        ht = htp.tile([P, Cs[0]], dt, tag="ht", name="ht")
                        nc.scalar.activation(
                            ht[:, :C], pa[:, :C],
                            mybir.ActivationFunctionType.Gelu,
                            bias=b1_t[:, e * KF + m: e * KF + m + 1])
                        hts.append(ht)
                        mt += tA[e] / KF
                    # phase B (transposed): pb[md] = sum_k w2_k.T @ ht_k
                    pbs = [psp.tile([P, 512], f32, tag="ps", name="pb")
                           for _md in range(MD)]
                    for k in range(KF):
                        emit_until(mt)
                        for md in range(MD):
                            nc.tensor.matmul(
                                pbs[md][:, :C],
                                w2ts[(e, k)][:, md * P:(md + 1) * P],
                                hts[k][:, :C],
                                start=(k == 0), stop=(k == KF - 1))
                        mt += tA[e] / KF
                    # drain: yT[md] = prob * pb[md]
                    for md in range(MD):
                        ot = outp.tile([P, 512], f32, tag="out", name="ot")
                        nc.vector.scalar_tensor_tensor(
                            ot[:, :C], pbs[md][:, :C], 1.0,
                            prb_t[0][:, offs[e]:offs[e] + C], mult, mult)
                        nc.scalar.dma_start(
                            y_d[e][md * P:(md + 1) * P, :], ot[:, :C])
    nc.finalize()
    return nc


def _route(x, gate_w, gate_b):
    """Top-2 routing on host. Returns flattened (expert, prob) per routed
    pair, the by-expert sort order, per-expert counts/starts, and each
    pair's position within its expert segment."""
    T = x.shape[0]
    scores = x @ gate_w + gate_b                      # [T, E]
    part = np.argpartition(scores, -TOP_K, axis=1)[:, -TOP_K:]   # [T, 2]
    vals = np.take_along_axis(scores, part, axis=1)
    vmax = vals.max(axis=1, keepdims=True)
    ex = np.exp(vals - vmax)
    prob = ex / ex.sum(axis=1, keepdims=True)

    expert_flat = part.ravel()                        # [2T]
    prob_flat = prob.ravel().astype(np.float32)
    token_flat = np.repeat(np.arange(T), TOP_K)

    order = np.argsort(expert_flat, kind="stable")
    counts = np.bincount(expert_flat, minlength=NUM_EXPERTS)
    starts = np.zeros(NUM_EXPERTS + 1, dtype=np.int64)
    np.cumsum(counts, out=starts[1:])

    inv_order = np.empty_like(order)
    inv_order[order] = np.arange(order.size)
    pos = inv_order - starts[expert_flat]
    return (expert_flat, prob_flat, token_flat, order, counts, starts, pos)


def _prepare(x, gate_w, gate_b, w1, b1, w2, b2):
    """Host-side routing, balanced expert->(core,slot) assignment, and
    per-core input packing. Returns (in_maps, Cs, meta-for-combine)."""
    B, S, D = x.shape
    T = B * S
    xf = np.ascontiguousarray(x.reshape(T, D), dtype=np.float32)

    (expert_flat, prob_flat, token_flat, order, counts, starts, pos) = _route(
        xf, np.asarray(gate_w, np.float32), np.asarray(gate_b, np.float32))

    # balanced assignment: slot j of core c holds expert_desc[j*8 + c]
    expert_desc = np.argsort(-counts, kind="stable")
    core_of = np.empty(NUM_EXPERTS, dtype=np.int64)
    slot_of = np.empty(NUM_EXPERTS, dtype=np.int64)
    for j in range(E_LOC):
        for c in range(N_CORES):
            e = expert_desc[j * N_CORES + c]
            core_of[e] = c
            slot_of[e] = j
    Cs = []
    for j in range(E_LOC):
        mx = int(counts[expert_desc[j * N_CORES:(j + 1) * N_CORES]].max())
        Cs.append(max(16, -(-mx // 16) * 16))        # 16-aligned exact cap
        assert Cs[j] <= 512
    CT = sum(Cs)
    offs = [sum(Cs[:j]) for j in range(E_LOC)]

    xg16 = xf[token_flat[order]].astype(BF16)         # [2T, D] sorted by expert
    sorted_probs = prob_flat[order]

    w1_16 = np.asarray(w1, np.float32).astype(BF16)   # [E, D, F]
    w2_16 = np.asarray(w2, np.float32).astype(BF16)   # [E, F, D]
    b1_f = np.asarray(b1, np.float32)                 # [E, F]

    in_maps = []
    for c in range(N_CORES):
        m = {}
        w1_core = np.empty((E_LOC, KD, W1C, P, W1CW), dtype=BF16)
        w2_core = np.empty((E_LOC, KF, P, D_MODEL), dtype=BF16)
        b1_core = np.empty((E_LOC, D_FF), dtype=np.float32)
        prb_core = np.zeros((P, CT), dtype=np.float32)
        for j in range(E_LOC):
            e = expert_desc[j * N_CORES + c]
            c_e = int(counts[e])
            xt_j = np.zeros((D, Cs[j]), dtype=BF16)
            if c_e:
                seg = slice(starts[e], starts[e] + c_e)
                xt_j[:, :c_e] = xg16[seg].T
                prb_core[:, offs[j]:offs[j] + c_e] = sorted_probs[seg][None, :]
            m[f"xt{j}"] = np.ascontiguousarray(xt_j.reshape(KD, P, Cs[j]))
            w1_core[j] = (w1_16[e].reshape(KD, P, W1C, W1CW)
                          .transpose(0, 2, 1, 3))
            w2_core[j] = w2_16[e].reshape(KF, P, D_MODEL)
            b1_core[j] = b1_f[e]
        m["w1"] = np.ascontiguousarray(w1_core)
        m["w2"] = np.ascontiguousarray(w2_core)
        m["prb"] = np.ascontiguousarray(prb_core)
        m["b1"] = np.ascontiguousarray(
            b1_core.reshape(E_LOC, KF, P).transpose(2, 0, 1)
            .reshape(P, E_LOC * KF))
        in_maps.append(m)

    meta = dict(T=T, shape=x.shape, CT=CT, offs=offs,
                core_of=core_of, slot_of=slot_of,
                expert_flat=expert_flat, prob_flat=prob_flat,
                token_flat=token_flat, pos=pos, b2=np.asarray(b2, np.float32))
    return in_maps, Cs, meta


def _combine(y_per_core, meta):
    """out[t] = sum of the token's two routed expert outputs (+ b2 term).
    Each per-core entry is [y0, y1, y2] with yj = [D_MODEL, C_j]."""
    T = meta["T"]
    CT = meta["CT"]
    offs = np.asarray(meta["offs"], dtype=np.int64)
    expert_flat = meta["expert_flat"]
    yt = np.concatenate(
        [np.concatenate(ys, axis=1) for ys in y_per_core], axis=1)  # [D, 8*CT]

    cols = (meta["core_of"][expert_flat] * CT
            + offs[meta["slot_of"][expert_flat]] + meta["pos"])
    cols = cols.reshape(T, TOP_K)
    out = (yt[:, cols[:, 0]] + yt[:, cols[:, 1]]).T    # [T, D]

    b2_f = meta["b2"]
    if np.any(b2_f):
        combine = np.zeros((T, NUM_EXPERTS), dtype=np.float32)
        np.add.at(combine, (meta["token_flat"], expert_flat), meta["prob_flat"])
        out = out + combine @ b2_f
    return np.ascontiguousarray(out.reshape(meta["shape"]), dtype=np.float32)


def kernel(x, gate_w, gate_b, w1, b1, w2, b2):
    from concourse import bass_utils

    in_maps, Cs, meta = _prepare(x, gate_w, gate_b, w1, b1, w2, b2)
    nc = _build(Cs)
    res = bass_utils.run_bass_kernel_spmd(nc, in_maps, core_ids=list(range(N_CORES)))
    return _combine([[res.results[c][f"y{j}"] for j in range(E_LOC)]
                     for c in range(N_CORES)], meta)
